# Initial kernel scaffold
#
"""YOLO DetectionLayer decode kernel for 8 Trainium2 NeuronCores.

Input  x [32, 255, 76, 76] fp32 -> output [32, 17328, 85] fp32.

Key layout fact: per image, out[(hw*3+box)*85 + attr] = f(x[box*85+attr, hw]),
i.e. the output is exactly the transpose of the [255, 5776] channel-major
input with per-channel activations (sigmoid / exp) and an affine box decode.

Per core (4 images): load [255,5776] channel-major, sigmoid in place,
TensorE-transpose 128-col chunks into PSUM, evacuate into a cell-major
SBUF staging tile (overwriting the 12 box-coord columns from a separately
computed "P12" tile holding x1y1 / x2y2 in channel-major), then store
contiguous [cells, 255] rows (= the exact output layout).

Sharding: pure data parallel, batch 32 -> 8 cores x 4 images.
"""
import sys

sys.path.insert(0, '/opt/trn_rl_repo')

import numpy as np

NCORES = 8
BPC = 4          # batch per core
NCH = 255
HW = 5776        # 76*76
NATT = 85
IMG = 608.0
XYS = 1.05
GRID = 76.0
ANCHOR_WH = np.array([[10.0, 13.0], [16.0, 30.0], [33.0, 23.0]], np.float32)

_CACHE = {}


def _legalize_waits(nc, mybir):
    """walrus core_v3 rejects >1 wait on most instructions (2 on
    EventSemaphore). Tile's final drain carries one wait per live semaphore;
    split the excess onto preceding EventSemaphore carrier instructions."""
    n_new = 0
    for func in nc.m.functions:
        for block in func.blocks:
            out, changed = [], False
            for inst in block.instructions:
                si = inst.sync_info
                if si is not None:
                    waits = list(si.on_wait or [])
                    cap = 2 if isinstance(inst, mybir.InstEventSemaphore) else 1
                    if len(waits) > cap:
                        keep, extra = waits[:cap], waits[cap:]
                        for i in range(0, len(extra), 2):
                            es = mybir.InstEventSemaphore(
                                name=f"{inst.name}-ws{i}", ins=[], outs=[])
                            es.engine = inst.engine
                            es.sync_info = mybir.SyncInfo(
                                on_wait=list(extra[i:i + 2]), on_update=[])
                            out.append(es)
                            n_new += 1
                        inst.sync_info = mybir.SyncInfo(
                            on_wait=keep, on_update=list(si.on_update or []))
                        changed = True
                out.append(inst)
            if changed:
                block.instructions[:] = out
    return n_new


def make_consts():
    """Host-precomputed constant tensors (identical on every core).

    Row layout of the 128-partition box-coord tiles:
      row = 32*b + dup*6 + box*2 + ch   (b image-in-core, dup 0:x1y1 1:x2y2,
                                         ch 0:x 1:y); rows r%32 >= 12 unused.
    """
    cell = np.arange(HW, dtype=np.float64)
    gx = (cell % 76 - 0.5 * (XYS - 1.0)) / GRID
    gy = (cell // 76 - 0.5 * (XYS - 1.0)) / GRID
    xyoff = np.zeros((128, HW), np.float32)
    anc = np.zeros((128, 1), np.float32)
    for b in range(BPC):
        for dup in range(2):
            for box in range(3):
                for ch in range(2):
                    r = 32 * b + dup * 6 + box * 2 + ch
                    xyoff[r] = (gx if ch == 0 else gy).astype(np.float32)
                    sgn = -1.0 if dup == 0 else 1.0
                    anc[r, 0] = sgn * ANCHOR_WH[box, ch] / (2.0 * IMG)
    return xyoff, anc


def _build(niter=1):
    import concourse.bass as bass
    import concourse.mybir as mybir
    from concourse.tile import TileContext
    from concourse import masks

    F32 = mybir.dt.float32
    AF = mybir.ActivationFunctionType

    nc = bass.Bass("TRN2")
    x = nc.dram_tensor("x", [BPC, NCH, 76, 76], F32, kind="ExternalInput")
    xyoff = nc.dram_tensor("xyoff", [128, HW], F32, kind="ExternalInput")
    anc = nc.dram_tensor("anc", [128, 1], F32, kind="ExternalInput")
    out = nc.dram_tensor("out", [BPC, HW * 3, NATT], F32, kind="ExternalOutput")

    xf = x[:].rearrange("b c h w -> b c (h w)")                  # [4,255,5776]
    xa = xf.rearrange("b (box a) hw -> b box a hw", box=3)       # [4,3,85,5776]
    out2 = out[:].rearrange("b r a -> b (r a)")                  # [4,1473840]

    with TileContext(nc) as tc:
        with tc.tile_pool(name="const", bufs=1) as cpool, \
             tc.tile_pool(name="p12", bufs=niter if niter > 1 else 1) as p12pool:
            ident = cpool.tile([128, 128], F32)
            masks.make_identity(nc, ident[:])
            anct = cpool.tile([128, 1], F32)
            nc.sync.dma_start(out=anct[:], in_=anc[:])

            for it in range(niter):
                # ---------------- box-coord precompute (P12) ----------------
                with tc.tile_pool(name="tmp", bufs=1) as tmp:
                    xyt = tmp.tile([128, HW], F32)
                    wht = tmp.tile([128, HW], F32)
                    xot = tmp.tile([128, HW], F32)
                    nc.sync.dma_start(out=xot[:], in_=xyoff[:])
                    for b in range(BPC):
                        for dup in range(2):
                            r0 = 32 * b + 6 * dup
                            nc.sync.dma_start(
                                out=xyt[r0:r0 + 6, :].rearrange(
                                    "(box a) c -> box a c", box=3),
                                in_=xa[b, :, 0:2, :])
                            nc.sync.dma_start(
                                out=wht[r0:r0 + 6, :].rearrange(
                                    "(box a) c -> box a c", box=3),
                                in_=xa[b, :, 2:4, :])
                    # image_wh/2 (signed): exp(wh) * (+-anchor/(2*608))
                    nc.scalar.activation(wht[:], wht[:], AF.Exp)
                    # image_xy: sigmoid(xy)*1.05/76 + (g - 0.025)/76
                    nc.scalar.activation(xyt[:], xyt[:], AF.Sigmoid)
                    nc.vector.tensor_scalar_mul(xyt[:], xyt[:], XYS / GRID)
                    nc.vector.tensor_add(xyt[:], xyt[:], xot[:])
                    nc.vector.tensor_scalar_mul(wht[:], wht[:], anct[:, 0:1])
                    p12 = p12pool.tile([128, HW], F32)
                    nc.vector.tensor_add(p12[:], xyt[:], wht[:])

                # ---------------- main per-image pipeline ----------------
                with tc.tile_pool(name="t0", bufs=2) as t0pool, \
                     tc.tile_pool(name="t1", bufs=2) as t1pool, \
                     tc.tile_pool(name="og", bufs=2) as ogpool, \
                     tc.tile_pool(name="ps0", bufs=2, space="PSUM") as ps0pool, \
                     tc.tile_pool(name="ps1", bufs=2, space="PSUM") as ps1pool, \
                     tc.tile_pool(name="psP", bufs=2, space="PSUM") as psPpool:
                    for b in range(BPC):
                        t0 = t0pool.tile([128, HW], F32)
                        t1 = t1pool.tile([127, HW], F32)
                        nc.sync.dma_start(out=t0[:], in_=xf[b, 0:128])
                        nc.sync.dma_start(out=t1[:], in_=xf[b, 128:255])
                        nc.scalar.activation(t0[:], t0[:], AF.Sigmoid)
                        nc.scalar.activation(t1[:], t1[:], AF.Sigmoid)
                        for og in range(6):            # 8-chunk output groups
                            j0, j1 = og * 8, min(og * 8 + 8, 46)
                            O = ogpool.tile([128, 2040], F32)
                            for g4 in range(j0, j1, 4):
                                jj = list(range(g4, min(g4 + 4, j1)))
                                n = len(jj)
                                ps0 = ps0pool.tile([128, 512], F32)
                                ps1 = ps1pool.tile([128, 512], F32)
                                psP = psPpool.tile([128, 512], F32)
                                for k, j in enumerate(jj):
                                    c0 = j * 128
                                    w = min(128, HW - c0)
                                    nc.tensor.transpose(
                                        ps0[:w, k * 128:k * 128 + 128],
                                        t0[:, c0:c0 + w], ident[:, :])
                                    nc.tensor.transpose(
                                        ps1[:w, k * 128:k * 128 + 127],
                                        t1[:, c0:c0 + w], ident[:127, :127])
                                    nc.tensor.transpose(
                                        psP[:w, k * 128:k * 128 + 128],
                                        p12[:, c0:c0 + w], ident[:, :])
                                m = g4 - og * 8
                                full = all(min(128, HW - j * 128) == 128
                                           for j in jj)
                                if full:
                                    o3 = O[:, m * 255:(m + n) * 255].rearrange(
                                        "p (k a) -> p k a", a=255)
                                    s0 = ps0[:, :n * 128].rearrange(
                                        "p (k a) -> p k a", a=128)
                                    s1 = ps1[:, :n * 128].rearrange(
                                        "p (k a) -> p k a", a=128)
                                    nc.scalar.copy(o3[:, :, 0:128], s0)
                                    nc.vector.tensor_copy(
                                        o3[:, :, 128:255], s1[:, :, 0:127])
                                    dst = O[:, m * 255:(m + n) * 255].rearrange(
                                        "p (k box r) -> p k box r", box=3
                                    )[:, :, :, 0:4].rearrange(
                                        "p k box (dup ch) -> p k box dup ch",
                                        dup=2)
                                    src = psP[:, :n * 128].rearrange(
                                        "p (k z) -> p k z", z=128
                                    )[:, :, 32 * b:32 * b + 12].rearrange(
                                        "p k (dup box ch) -> p k box dup ch",
                                        dup=2, box=3)
                                    nc.vector.tensor_copy(dst, src)
                                else:
                                    for k, j in enumerate(jj):
                                        w = min(128, HW - j * 128)
                                        ok = O[:, (m + k) * 255:(m + k + 1) * 255]
                                        nc.scalar.copy(
                                            ok[:w, 0:128],
                                            ps0[:w, k * 128:k * 128 + 128])
                                        nc.vector.tensor_copy(
                                            ok[:w, 128:255],
                                            ps1[:w, k * 128:k * 128 + 127])
                                        dst = ok[:w, :].rearrange(
                                            "p (box r) -> p box r", box=3
                                        )[:, :, 0:4].rearrange(
                                            "p box (dup ch) -> p box dup ch",
                                            dup=2)
                                        src = psP[:w, k * 128 + 32 * b:
                                                  k * 128 + 32 * b + 12].rearrange(
                                            "p (dup box ch) -> p box dup ch",
                                            dup=2, box=3)
                                        nc.vector.tensor_copy(dst, src)
                            # store this output group
                            cell0 = og * 1024
                            if og < 5:
                                dst = out2[b, cell0 * 255:(cell0 + 1024) * 255
                                           ].rearrange("(k p a) -> p k a",
                                                       p=128, a=255)
                                nc.sync.dma_start(
                                    out=dst,
                                    in_=O[:].rearrange("p (k a) -> p k a",
                                                       a=255))
                            else:
                                dst = out2[b, cell0 * 255:(cell0 + 640) * 255
                                           ].rearrange("(k p a) -> p k a",
                                                       p=128, a=255)
                                nc.sync.dma_start(
                                    out=dst,
                                    in_=O[:, :5 * 255].rearrange(
                                        "p (k a) -> p k a", a=255))
                                dst2 = out2[b, 5760 * 255:5776 * 255
                                            ].rearrange("(p a) -> p a", a=255)
                                nc.sync.dma_start(
                                    out=dst2, in_=O[0:16, 5 * 255:6 * 255])

    _legalize_waits(nc, mybir)
    return nc


def _get_built(niter=1):
    if niter not in _CACHE:
        _CACHE[niter] = _build(niter)
    return _CACHE[niter]


def run_on_cores(x, niter=1):
    from concourse import bass_utils
    nc = _get_built(niter)
    xyoff, anc = make_consts()
    x8 = np.ascontiguousarray(np.asarray(x, np.float32).reshape(
        NCORES, BPC, NCH, 76, 76))
    in_maps = [{"x": x8[i], "xyoff": xyoff, "anc": anc}
               for i in range(NCORES)]
    res = bass_utils.run_bass_kernel_spmd(nc, in_maps,
                                          core_ids=list(range(NCORES)))
    outs = np.stack([res.results[i]["out"] for i in range(NCORES)])
    return outs.reshape(NCORES * BPC, HW * 3, NATT)


def kernel(x):
    return run_on_cores(x, niter=1)


# revision 4
# speedup vs baseline: 16.3046x; 16.3046x over previous
"""YOLO DetectionLayer decode kernel for 8 Trainium2 NeuronCores.

Input  x [32, 255, 76, 76] fp32 -> output [32, 17328, 85] fp32.

Key layout fact: per image, out[(hw*3+box)*85 + attr] = f(x[box*85+attr, hw]),
i.e. the output is exactly the transpose of the [255, 5776] channel-major
input with per-channel activations (sigmoid / exp) and an affine box decode.

Per core (4 images): load [255,5776] channel-major, sigmoid in place,
TensorE-transpose 128-col chunks into PSUM, evacuate into a cell-major
SBUF staging tile (overwriting the 12 box-coord columns from a separately
computed "P12" tile holding x1y1 / x2y2 in channel-major), then store
contiguous [cells, 255] rows (= the exact output layout).

Sharding: pure data parallel, batch 32 -> 8 cores x 4 images.
"""
import sys

sys.path.insert(0, '/opt/trn_rl_repo')

import numpy as np

NCORES = 8
BPC = 4          # batch per core
NCH = 255
HW = 5776        # 76*76
NATT = 85
IMG = 608.0
XYS = 1.05
GRID = 76.0
ANCHOR_WH = np.array([[10.0, 13.0], [16.0, 30.0], [33.0, 23.0]], np.float32)

_CACHE = {}


def _legalize_waits(nc, mybir):
    """walrus core_v3 rejects >1 wait on most instructions (2 on
    EventSemaphore). Tile's final drain carries one wait per live semaphore;
    split the excess onto preceding EventSemaphore carrier instructions."""
    n_new = 0
    for func in nc.m.functions:
        for block in func.blocks:
            out, changed = [], False
            for inst in block.instructions:
                si = inst.sync_info
                if si is not None:
                    waits = list(si.on_wait or [])
                    cap = 2 if isinstance(inst, mybir.InstEventSemaphore) else 1
                    if len(waits) > cap:
                        keep, extra = waits[:cap], waits[cap:]
                        for i in range(0, len(extra), 2):
                            es = mybir.InstEventSemaphore(
                                name=f"{inst.name}-ws{i}", ins=[], outs=[])
                            es.engine = inst.engine
                            es.sync_info = mybir.SyncInfo(
                                on_wait=list(extra[i:i + 2]), on_update=[])
                            out.append(es)
                            n_new += 1
                        inst.sync_info = mybir.SyncInfo(
                            on_wait=keep, on_update=list(si.on_update or []))
                        changed = True
                out.append(inst)
            if changed:
                block.instructions[:] = out
    return n_new


def make_consts():
    """Host-precomputed constant tensors (identical on every core).

    Row layout of the 128-partition box-coord tiles:
      row = 32*b + dup*6 + box*2 + ch   (b image-in-core, dup 0:x1y1 1:x2y2,
                                         ch 0:x 1:y); rows r%32 >= 12 unused.
    """
    cell = np.arange(HW, dtype=np.float64)
    gx = (cell % 76 - 0.5 * (XYS - 1.0)) / GRID
    gy = (cell // 76 - 0.5 * (XYS - 1.0)) / GRID
    xyoff = np.zeros((128, HW), np.float32)
    anc = np.zeros((128, 1), np.float32)
    for b in range(BPC):
        for dup in range(2):
            for box in range(3):
                for ch in range(2):
                    r = 32 * b + dup * 6 + box * 2 + ch
                    xyoff[r] = (gx if ch == 0 else gy).astype(np.float32)
                    sgn = -1.0 if dup == 0 else 1.0
                    anc[r, 0] = sgn * ANCHOR_WH[box, ch] / (2.0 * IMG)
    return xyoff, anc


def _build(niter=1):
    import concourse.bass as bass
    import concourse.mybir as mybir
    from concourse.tile import TileContext
    from concourse import masks

    F32 = mybir.dt.float32
    AF = mybir.ActivationFunctionType

    nc = bass.Bass("TRN2")
    x = nc.dram_tensor("x", [BPC, NCH, 76, 76], F32, kind="ExternalInput")
    xyoff = nc.dram_tensor("xyoff", [128, HW], F32, kind="ExternalInput")
    anc = nc.dram_tensor("anc", [128, 1], F32, kind="ExternalInput")
    out = nc.dram_tensor("out", [BPC, HW * 3, NATT], F32, kind="ExternalOutput")

    xf = x[:].rearrange("b c h w -> b c (h w)")                  # [4,255,5776]
    xa = xf.rearrange("b (box a) hw -> b box a hw", box=3)       # [4,3,85,5776]
    out2 = out[:].rearrange("b r a -> b (r a)")                  # [4,1473840]

    with TileContext(nc) as tc:
        with tc.tile_pool(name="const", bufs=1) as cpool, \
             tc.tile_pool(name="p12", bufs=min(niter, 2)) as p12pool:
            ident = cpool.tile([128, 128], F32)
            masks.make_identity(nc, ident[:])
            anct = cpool.tile([128, 1], F32)
            nc.sync.dma_start(out=anct[:], in_=anc[:])

            for it in range(niter):
                # ---------------- box-coord precompute (P12) ----------------
                with tc.tile_pool(name="tmp", bufs=1) as tmp:
                    xyt = tmp.tile([128, HW], F32)
                    wht = tmp.tile([128, HW], F32)
                    xot = tmp.tile([128, HW], F32)
                    nc.sync.dma_start(out=xot[:], in_=xyoff[:])
                    for b in range(BPC):
                        for dup in range(2):
                            r0 = 32 * b + 6 * dup
                            # dst stays a plain [6, HW] partition slice: a
                            # rearranged dst lets the AP optimizer merge
                            # partition+free dims, which HW descriptor
                            # generation then mislowers (sprays bytes across
                            # neighboring tiles). dma_start only checks total
                            # size, so the nested DRAM src pairs fine.
                            nc.sync.dma_start(
                                out=xyt[r0:r0 + 6, :], in_=xa[b, :, 0:2, :])
                            nc.sync.dma_start(
                                out=wht[r0:r0 + 6, :], in_=xa[b, :, 2:4, :])
                    # image_wh/2 (signed): exp(wh) * (+-anchor/(2*608))
                    nc.scalar.activation(wht[:], wht[:], AF.Exp)
                    # image_xy: sigmoid(xy)*1.05/76 + (g - 0.025)/76
                    nc.scalar.activation(xyt[:], xyt[:], AF.Sigmoid)
                    nc.vector.tensor_scalar_mul(xyt[:], xyt[:], XYS / GRID)
                    nc.vector.tensor_add(xyt[:], xyt[:], xot[:])
                    nc.vector.tensor_scalar_mul(wht[:], wht[:], anct[:, 0:1])
                    p12 = p12pool.tile([128, HW], F32)
                    nc.vector.tensor_add(p12[:], xyt[:], wht[:])

                # ---------------- main per-image pipeline ----------------
                with tc.tile_pool(name="t0", bufs=2) as t0pool, \
                     tc.tile_pool(name="t1", bufs=2) as t1pool, \
                     tc.tile_pool(name="og", bufs=2) as ogpool, \
                     tc.tile_pool(name="ps0", bufs=2, space="PSUM") as ps0pool, \
                     tc.tile_pool(name="ps1", bufs=2, space="PSUM") as ps1pool, \
                     tc.tile_pool(name="psP", bufs=2, space="PSUM") as psPpool:
                    for b in range(BPC):
                        t0 = t0pool.tile([128, HW], F32)
                        t1 = t1pool.tile([127, HW], F32)
                        nc.sync.dma_start(out=t0[:], in_=xf[b, 0:128])
                        nc.sync.dma_start(out=t1[:], in_=xf[b, 128:255])
                        nc.scalar.activation(t0[:], t0[:], AF.Sigmoid)
                        nc.scalar.activation(t1[:], t1[:], AF.Sigmoid)
                        for og in range(6):            # 8-chunk output groups
                            j0, j1 = og * 8, min(og * 8 + 8, 46)
                            O = ogpool.tile([128, 2040], F32)
                            for g4 in range(j0, j1, 4):
                                jj = list(range(g4, min(g4 + 4, j1)))
                                n = len(jj)
                                ps0 = ps0pool.tile([128, 512], F32)
                                ps1 = ps1pool.tile([128, 512], F32)
                                psP = psPpool.tile([128, 512], F32)
                                for k, j in enumerate(jj):
                                    c0 = j * 128
                                    w = min(128, HW - c0)
                                    nc.tensor.transpose(
                                        ps0[:w, k * 128:k * 128 + 128],
                                        t0[:, c0:c0 + w], ident[:, :])
                                    nc.tensor.transpose(
                                        ps1[:w, k * 128:k * 128 + 127],
                                        t1[:, c0:c0 + w], ident[:127, :127])
                                    nc.tensor.transpose(
                                        psP[:w, k * 128:k * 128 + 128],
                                        p12[:, c0:c0 + w], ident[:, :])
                                m = g4 - og * 8
                                full = all(min(128, HW - j * 128) == 128
                                           for j in jj)
                                if full:
                                    o3 = O[:, m * 255:(m + n) * 255].rearrange(
                                        "p (k a) -> p k a", a=255)
                                    s0 = ps0[:, :n * 128].rearrange(
                                        "p (k a) -> p k a", a=128)
                                    s1 = ps1[:, :n * 128].rearrange(
                                        "p (k a) -> p k a", a=128)
                                    nc.scalar.copy(o3[:, :, 0:128], s0)
                                    nc.vector.tensor_copy(
                                        o3[:, :, 128:255], s1[:, :, 0:127])
                                    dst = O[:, m * 255:(m + n) * 255].rearrange(
                                        "p (k box r) -> p k box r", box=3, r=85
                                    )[:, :, :, 0:4].rearrange(
                                        "p k box (dup ch) -> p k box dup ch",
                                        dup=2)
                                    src = psP[:, :n * 128].rearrange(
                                        "p (k z) -> p k z", z=128
                                    )[:, :, 32 * b:32 * b + 12].rearrange(
                                        "p k (dup box ch) -> p k box dup ch",
                                        dup=2, box=3)
                                    nc.vector.tensor_copy(dst, src)
                                else:
                                    for k, j in enumerate(jj):
                                        w = min(128, HW - j * 128)
                                        ok = O[:, (m + k) * 255:(m + k + 1) * 255]
                                        nc.scalar.copy(
                                            ok[:w, 0:128],
                                            ps0[:w, k * 128:k * 128 + 128])
                                        nc.vector.tensor_copy(
                                            ok[:w, 128:255],
                                            ps1[:w, k * 128:k * 128 + 127])
                                        dst = ok[:w, :].rearrange(
                                            "p (box r) -> p box r", box=3, r=85
                                        )[:, :, 0:4].rearrange(
                                            "p box (dup ch) -> p box dup ch",
                                            dup=2)
                                        src = psP[:w, k * 128 + 32 * b:
                                                  k * 128 + 32 * b + 12].rearrange(
                                            "p (dup box ch) -> p box dup ch",
                                            dup=2, box=3)
                                        nc.vector.tensor_copy(dst, src)
                            # store this output group
                            cell0 = og * 1024
                            if og < 5:
                                dst = out2[b, cell0 * 255:(cell0 + 1024) * 255
                                           ].rearrange("(k p a) -> p k a",
                                                       p=128, a=255)
                                nc.sync.dma_start(
                                    out=dst,
                                    in_=O[:].rearrange("p (k a) -> p k a",
                                                       a=255))
                            else:
                                dst = out2[b, cell0 * 255:(cell0 + 640) * 255
                                           ].rearrange("(k p a) -> p k a",
                                                       p=128, a=255)
                                nc.sync.dma_start(
                                    out=dst,
                                    in_=O[:, :5 * 255].rearrange(
                                        "p (k a) -> p k a", a=255))
                                dst2 = out2[b, 5760 * 255:5776 * 255
                                            ].rearrange("(p a) -> p a", a=255)
                                nc.sync.dma_start(
                                    out=dst2, in_=O[0:16, 5 * 255:6 * 255])

    _legalize_waits(nc, mybir)
    return nc


def _get_built(niter=1):
    if niter not in _CACHE:
        _CACHE[niter] = _build(niter)
    return _CACHE[niter]


def run_on_cores(x, niter=1):
    from concourse import bass_utils
    nc = _get_built(niter)
    xyoff, anc = make_consts()
    x8 = np.ascontiguousarray(np.asarray(x, np.float32).reshape(
        NCORES, BPC, NCH, 76, 76))
    in_maps = [{"x": x8[i], "xyoff": xyoff, "anc": anc}
               for i in range(NCORES)]
    res = bass_utils.run_bass_kernel_spmd(nc, in_maps,
                                          core_ids=list(range(NCORES)))
    outs = np.stack([res.results[i]["out"] for i in range(NCORES)])
    return outs.reshape(NCORES * BPC, HW * 3, NATT)


def kernel(x):
    return run_on_cores(x, niter=1)
